# revision 5
# baseline (speedup 1.0000x reference)
"""Trainium2 Bass kernel for the 4-layer sum/product circuit (segment_reduce).

Strategy: shard batch (4096) across 8 cores (512 each), zero communication.
Node-major layout: every circuit array lives in HBM as [n_nodes, 512] rows
(one row = one node's batch slice, 2KB fp32 / 1KB bf16). Each layer is a
chunked SWDGE dma_gather (16 SDMA engines, ~360GB/s) with host-permuted
indices so the k legs of each output land in contiguous free-axis columns
of one partition; DVE does the k-leg sums (fp32 accum), ACT applies
exp/ln, HWDGE writes the chunk back node-major. Prob-domain intermediates
(e1, e3) are stored bf16 to halve their gather volume; log-domain arrays
(xenc, l2) stay fp32. GPSIMD only generates DMA descriptors.
"""

import math
import numpy as np
from contextlib import ExitStack

import concourse.bacc as bacc
import concourse.tile as tile
from concourse import bass, mybir
from concourse import library_config
from concourse.bass_utils import run_bass_kernel_spmd

N_CORES = 8
B = 4096
ELEM = B // N_CORES         # 512 batch per core = one gathered row

N_XENC = 2050
N_L1 = 8192
N_L2 = 4096
N_L3 = 8192
N_OUT = 2048

CHUNK_IDX = 1024            # gather indices per dma_gather (HW limit ~1024)

_EXP = mybir.ActivationFunctionType.Exp
_LN = mybir.ActivationFunctionType.Ln
_FP32 = mybir.dt.float32
_BF16 = mybir.dt.bfloat16

# (idx_name, src_name, n_src, n_out, k, act, out_name, out_dtype)
LAYERS = [
    ("g1", "xenc", N_XENC, N_L1, 4, _EXP, "e1", _BF16),
    ("g2", "e1", N_L1, N_L2, 8, _LN, "l2", _FP32),
    ("g3", "l2", N_L2, N_L3, 4, _EXP, "e3", _BF16),
    ("g4", "e3", N_L3, N_OUT, 8, _LN, "out", _FP32),
]


def _perm_wrap_idx(orig: np.ndarray, n_out: int, k: int) -> np.ndarray:
    """Permute [n_out, k] gather indices into dma_gather order and wrap.

    dma_gather writes gathered row i to (partition i%128, col i//128). We
    want output o's leg j at (p = o%128, col = (o//128)*k + j), i.e.
    i = ((o//128)*k + j)*128 + o%128, so the k legs of each output are
    contiguous columns within one partition.
    Returns the int16 [128, n_out*k//16] wrapped index tile.
    """
    og = orig.reshape(n_out // 128, 128, k)         # [o2, p, j]
    flat = og.transpose(0, 2, 1).reshape(-1)        # [(o2 k) p] -> i = c*128+p
    q = flat.shape[0]
    w = flat.reshape(q // 16, 16).T.astype(np.int16)  # [16, q/16]
    return np.tile(w, (8, 1))                       # [128, q/16]


def _log1mexp(x):
    # match reference (Maechler 2012) in f32
    x = x.astype(np.float32)
    with np.errstate(divide="ignore", invalid="ignore"):
        a = np.log(-np.expm1(x)).astype(np.float32)
        b = np.log1p(-np.exp(x)).astype(np.float32)
    return np.where(x > -math.log(2.0), a, b).astype(np.float32)


def _build(nc):
    i16 = mybir.dt.int16
    add = mybir.AluOpType.add

    xenc_d = nc.dram_tensor("xenc", [N_XENC, ELEM], _FP32,
                            kind="ExternalInput")
    e1_d = nc.dram_tensor("e1", [N_L1, ELEM], _BF16, kind="Internal")
    l2_d = nc.dram_tensor("l2", [N_L2, ELEM], _FP32, kind="Internal")
    e3_d = nc.dram_tensor("e3", [N_L3, ELEM], _BF16, kind="Internal")
    out_d = nc.dram_tensor("out", [N_OUT, ELEM], _FP32,
                           kind="ExternalOutput")
    tensors = {"xenc": xenc_d, "e1": e1_d, "l2": l2_d, "e3": e3_d,
               "out": out_d}

    idx_d = {}
    for name, _, _, n_out, k, _, _, _ in LAYERS:
        idx_d[name] = nc.dram_tensor(f"{name}idx", [128, n_out * k // 16],
                                     i16, kind="ExternalInput").ap()

    with tile.TileContext(nc) as tc, ExitStack() as ctx:
        nc.gpsimd.load_library(library_config.mlp)
        idxp = ctx.enter_context(tc.tile_pool(name="idxp", bufs=1))
        gpool = ctx.enter_context(tc.tile_pool(name="gpool", bufs=2))
        apool = ctx.enter_context(tc.tile_pool(name="apool", bufs=2))
        rpool = ctx.enter_context(tc.tile_pool(name="rpool", bufs=2))

        for idx_name, src_name, n_src, n_out, k, act, dst_name, out_dt \
                in LAYERS:
            src_dt = tensors[src_name].dtype
            src_ap = tensors[src_name].ap()
            idx_t = idxp.tile(list(idx_d[idx_name].shape), i16, tag="idx")
            nc.sync.dma_start(idx_t[:], idx_d[idx_name][:])

            chunk_out = CHUNK_IDX // k              # output nodes per chunk
            oc = chunk_out // 128                   # output cols per chunk
            gcols = CHUNK_IDX // 128                # gather cols per chunk
            icols = CHUNK_IDX // 16                 # idx cols per chunk
            chunk_idx = CHUNK_IDX
            nchunks = n_out // chunk_out
            dst_view = tensors[dst_name].ap().rearrange(
                "(c o p) e -> c p o e", p=128, o=oc)

            for ci in range(nchunks):
                g = gpool.tile([128, gcols, ELEM], src_dt, tag="g")
                nc.gpsimd.dma_gather(
                    g[:], src_ap,
                    idx_t[:, ci * icols:(ci + 1) * icols],
                    chunk_idx, chunk_idx, ELEM,
                )
                ga = g[:].rearrange("p (o k) e -> p o k e", k=k)
                acc = apool.tile([128, oc, ELEM], _FP32, tag="acc")
                nc.vector.tensor_tensor(acc[:], ga[:, :, 0, :],
                                        ga[:, :, 1, :], add)
                for j in range(2, k):
                    nc.vector.tensor_tensor(acc[:], acc[:], ga[:, :, j, :],
                                            add)
                r = rpool.tile([128, oc, ELEM], out_dt, tag="r")
                nc.scalar.activation(r[:], acc[:], act)
                nc.sync.dma_start(dst_view[ci], r[:])
    nc.compile()
    return nc


_CACHED_NC = None
_LAST_IN_MAPS = None


def kernel(pos, idx0, idx1, idx2, idx3):
    global _CACHED_NC, _LAST_IN_MAPS
    pos = np.asarray(pos, dtype=np.float32)

    # host-side input encoding: x_enc [2050, 4096]
    neg = _log1mexp(pos)
    n, b = pos.shape
    xenc = np.zeros((2 * n + 2, b), np.float32)
    xenc[1] = 0.0
    xenc[2::2] = pos
    xenc[3::2] = neg
    # row 0 is -inf in the reference but never gathered (idx0 >= 1); keep 0.

    idx_maps = {}
    for (name, _, _, n_out, k, _, _, _), arr in zip(
            LAYERS, (idx0, idx1, idx2, idx3)):
        idx_maps[f"{name}idx"] = _perm_wrap_idx(
            np.asarray(arr).astype(np.int64), n_out, k)

    if _CACHED_NC is None:
        _CACHED_NC = _build(bacc.Bacc("TRN2", target_bir_lowering=False,
                                      debug=False,
                                      dynamic_dma_scratch_size=32768))
    nc = _CACHED_NC

    in_maps = []
    for c in range(N_CORES):
        sl = np.ascontiguousarray(xenc[:, c * ELEM:(c + 1) * ELEM])
        in_maps.append({"xenc": sl, **idx_maps})

    _LAST_IN_MAPS = in_maps
    res = run_bass_kernel_spmd(nc, in_maps, list(range(N_CORES)))
    out = np.empty((N_OUT, B), np.float32)
    for c in range(N_CORES):
        out[:, c * ELEM:(c + 1) * ELEM] = res.results[c]["out"]
    return out
